# revision 2
# baseline (speedup 1.0000x reference)
"""Self-contained 8-core Trainium2 Bass kernel for MultiHeadAttention.

Problem: B=2, S=2048, D=1024, H=16 heads (hd=64), f32, self-attention
(no mask), eval mode (dropout = identity).

Sharding: data-parallel over B (2) x tensor-parallel over heads (4 groups
of 4 heads) = 8 cores. Each core computes, for its batch b and its 4
heads: Q/K/V projections (column-sliced), attention, and a partial
output projection (row-sliced Wo). Host sums the 4 partials per batch
and adds the (bv @ Wo + bo) correction (bv never enters the kernel:
ctx rows sum probs to 1, so (ctx+bv) @ Wo = ctx @ Wo + bv @ Wo).

Algebraic simplifications used (exact):
  - bk dropped: softmax over k is invariant to the per-q constant Q.bk.
  - softmax computed without max subtraction (scores bounded ~|s|<10,
    exp is safe in f32).
  - bq folded into Q^T as a per-partition bias.
  - row normalization deferred past the P@V matmul (scale ctx instead
    of probs); row sums obtained free via an appended ones-column in V.

Performance notes (v2):
  - all matmul operands are bf16: f32r streams the PE moving operand at
    0.5 col/cycle, bf16 at 1 col/cycle -> 2x on every matmul. PSUM
    accumulation stays f32. Softmax numerator and denominator are
    computed from the same bf16 exp values, so normalization error
    largely cancels.
  - scores computed per head-pair as two concurrent K=64 row-tiled
    matmuls (tile_position (0,0)/(64,0)) -> ~2x over sequential.
  - exp on ACT is then the bottleneck (128 x [128,1024] tiles ~ 142us),
    so the schedule starts attention as early as possible (kt/qt of
    pair 0 first; V projection interleaved into the first attention
    block's r-loop) and keeps ACT busy back-to-back.

Layouts on chip (per core):
  - x^T [D, S] (host-transposed, bf16), Q^T/K^T [head-pair(128), S]
    with the two heads of a pair stacked on partitions -> scores^T
    computed as K @ Q^T with k-positions on the output partitions
    (softmax reductions become PE-contractions).
  - exp on ACT over 2-bank PSUM regions, output bf16.
  - PV: ctx^T[hd+1, q] = [V_h | 1]^T_k-major @ exp^T, accumulated over
    k-tiles in PSUM; row 64 is the softmax denominator.
"""

import sys

sys.path.insert(0, "/opt/trn_rl_repo")

import numpy as np

B, S, D, H, HD = 2, 2048, 1024, 16, 64
HPC = 4  # heads per core
NCORES = 8
DC = D // 128  # 8 contraction chunks
ST = S // 128  # 16 s-tiles
QCW = 512  # q chunk width
QC = S // QCW  # 4 q chunks
KT = S // 128  # 16 k tiles

_CACHE = {}


def _build(repeat=1, ep_bufs=4, ctx_bufs=2, qkv_bufs=2, mp_bufs=2, op_bufs=2):
    import concourse.bass as bass  # noqa: F401
    import concourse.mybir as mybir
    import concourse.tile as tile
    from concourse import bacc
    from concourse.library_config import attn as attn_lib

    F32 = mybir.dt.float32
    BF16 = mybir.dt.bfloat16
    AF = mybir.ActivationFunctionType

    nc = bacc.Bacc("TRN2", target_bir_lowering=False, debug=False)

    xt_d = nc.dram_tensor("xt", [D, S], BF16, kind="ExternalInput")
    wq_d = nc.dram_tensor("wq", [D, HPC * HD], BF16, kind="ExternalInput")
    wk_d = nc.dram_tensor("wk", [D, HPC * HD], BF16, kind="ExternalInput")
    wv_d = nc.dram_tensor("wv", [D, HPC * HD], BF16, kind="ExternalInput")
    wo_d = nc.dram_tensor("wo", [HPC * HD, D], BF16, kind="ExternalInput")
    bq_d = nc.dram_tensor("bq2", [128, 2], F32, kind="ExternalInput")
    out_d = nc.dram_tensor("out_p", [S, D], F32, kind="ExternalOutput")

    with tile.TileContext(nc) as tc:
        nc.gpsimd.load_library(attn_lib)
        with (
            tc.tile_pool(name="wp", bufs=1) as wp,
            tc.tile_pool(name="xp", bufs=1) as xp,
            tc.tile_pool(name="qk", bufs=1) as qk,
            tc.tile_pool(name="vp", bufs=1) as vp,
            tc.tile_pool(name="ep", bufs=ep_bufs) as ep,
            tc.tile_pool(name="cp", bufs=1) as cp,
            tc.tile_pool(name="mp", bufs=mp_bufs) as mp,
            tc.tile_pool(name="op", bufs=op_bufs) as op,
            tc.tile_pool(name="pp", bufs=2, space="PSUM") as pp,
        ):
            # ---- loads, ordered so pair-0 kt/qt can start earliest
            wk_t = wp.tile([128, DC, HPC * HD], BF16, tag="wk")
            nc.sync.dma_start(wk_t[:], wk_d.rearrange("(c p) n -> p c n", p=128))
            wq_t = wp.tile([128, DC, HPC * HD], BF16, tag="wq")
            nc.sync.dma_start(wq_t[:], wq_d.rearrange("(c p) n -> p c n", p=128))
            bq_t = wp.tile([128, 2], F32, tag="bq")
            nc.sync.dma_start(bq_t[:], bq_d[:])
            wv_t = wp.tile([128, DC, HPC * HD], BF16, tag="wv")
            nc.sync.dma_start(wv_t[:], wv_d.rearrange("(c p) n -> p c n", p=128))
            # xt in q-quarter-major chunks so kt(0,qc)/v_proj can start
            # before the full 4MB lands
            xt_t = xp.tile([128, DC, S], BF16, tag="xt")
            for qc in range(QC):
                qs = slice(qc * QCW, (qc + 1) * QCW)
                for c in range(DC):
                    nc.sync.dma_start(xt_t[:, c, qs], xt_d[c * 128:(c + 1) * 128, qs])
            wo_t = wp.tile([128, 2, D], BF16, tag="wo")
            nc.sync.dma_start(wo_t[:], wo_d.rearrange("(c p) n -> p c n", p=128))
            ones_f = wp.tile([128, 64], BF16, tag="onesf")
            nc.vector.memset(ones_f[:], 1.0)

            import contextlib
            if repeat > 1:
                _engs = [mybir.EngineType.PE, mybir.EngineType.Activation,
                         mybir.EngineType.DVE, mybir.EngineType.SP,
                         mybir.EngineType.Pool]
                rep_ctx = tc.For_i(0, repeat, hint_engines=_engs, staggered_reset=True)
            else:
                rep_ctx = contextlib.nullcontext()
            with rep_ctx:
                # ---- V projection -> v1 [s, 4*(64+1)] with ones columns
                v1_t = vp.tile([128, ST, HPC * 65], BF16, tag="v1")
                with nc.allow_low_precision(reason="bf16 matmul operands"):
                    nc.vector.tensor_copy(
                        v1_t[:].rearrange("p s (h c) -> p s h c", c=65)[:, :, :, 64],
                        ones_f[:, 0:64].rearrange("p (s h) -> p s h", s=ST),
                    )

                def v_proj(st):
                    vps = pp.tile([128, HPC * HD], F32, tag="qkv", bufs=qkv_bufs, name="vps")
                    for c in range(DC):
                        nc.tensor.matmul(
                            vps[:],
                            xt_t[:, c, st * 128:(st + 1) * 128],
                            wv_t[:, c, :],
                            start=(c == 0),
                            stop=(c == DC - 1),
                        )
                    with nc.allow_low_precision(reason="bf16 matmul operands"):
                        nc.vector.tensor_copy(
                            v1_t[:, st, :].rearrange("p (h c) -> p h c", c=65)[:, :, 0:64],
                            vps[:].rearrange("p (h c) -> p h c", c=64),
                        )

                # ---- Q^T / K^T projections (per head pair, 2 heads packed
                # on partitions)
                qt_tiles = [qk.tile([128, S], BF16, tag=f"qt{p}", name=f"qt{p}") for p in range(2)]
                kt_tiles = [qk.tile([128, S], BF16, tag=f"kt{p}", name=f"kt{p}") for p in range(2)]

                def kt_proj(pair, qc):
                    qs = slice(qc * QCW, (qc + 1) * QCW)
                    kps = pp.tile([128, QCW], F32, tag="qkv", bufs=qkv_bufs, name="kps")
                    for c in range(DC):
                        nc.tensor.matmul(
                            kps[:],
                            wk_t[:, c, pair * 128:(pair + 1) * 128],
                            xt_t[:, c, qs],
                            start=(c == 0),
                            stop=(c == DC - 1),
                        )
                    with nc.allow_low_precision(reason="bf16 score operands"):
                        nc.vector.tensor_copy(kt_tiles[pair][:, qs], kps[:])

                def qt_proj(pair, qc):
                    qs = slice(qc * QCW, (qc + 1) * QCW)
                    qps = pp.tile([128, QCW], F32, tag="qkv", bufs=qkv_bufs, name="qps")
                    for c in range(DC):
                        nc.tensor.matmul(
                            qps[:],
                            wq_t[:, c, pair * 128:(pair + 1) * 128],
                            xt_t[:, c, qs],
                            start=(c == 0),
                            stop=(c == DC - 1),
                        )
                    with nc.allow_low_precision(reason="bf16 score operands"):
                        nc.vector.tensor_scalar_add(
                            qt_tiles[pair][:, qs], qps[:], bq_t[:, pair:pair + 1]
                        )

                ctxt_tiles = [cp.tile([128, S], BF16, tag=f"ct{p}", name=f"ct{p}") for p in range(2)]

                def attention(pair, qc, vproj_from=None, vproj_n=0):
                    qs = slice(qc * QCW, (qc + 1) * QCW)
                    ctx_ps = [pp.tile([65, QCW], F32, tag="ctx", name=f"ctx{_h}", bufs=ctx_bufs) for _h in range(2)]
                    for r in range(KT):
                        if vproj_from is not None and r < vproj_n:
                            v_proj(vproj_from + r)
                        sreg = pp.tile([128, 2 * QCW], F32, tag="big")
                        expt = ep.tile([128, 2 * QCW], BF16, tag="exp")
                        for h in range(2):
                            nc.tensor.matmul(
                                sreg[:, h * QCW:(h + 1) * QCW],
                                kt_tiles[pair][64 * h:64 * (h + 1), r * 128:(r + 1) * 128],
                                qt_tiles[pair][64 * h:64 * (h + 1), qs],
                                start=True,
                                stop=True,
                                tile_position=(64 * h, 0),
                            )
                        with nc.allow_low_precision(reason="bf16 probs"):
                            nc.scalar.activation(expt[:], sreg[:], AF.Exp, scale=0.125)
                        for h in range(2):
                            hh = 2 * pair + h
                            nc.tensor.matmul(
                                ctx_ps[h][:],
                                v1_t[:, r, 65 * hh:65 * hh + 65],
                                expt[:, h * QCW:(h + 1) * QCW],
                                start=(r == 0),
                                stop=(r == KT - 1),
                            )
                    for h in range(2):
                        rsum = mp.tile([1, QCW], F32, tag="rsum")
                        nc.vector.reciprocal(rsum[:], ctx_ps[h][64:65, :])
                        bct = mp.tile([64, QCW], F32, tag="bc")
                        nc.gpsimd.partition_broadcast(bct[:], rsum[:])
                        with nc.allow_low_precision(reason="bf16 matmul operands"):
                            nc.vector.tensor_mul(
                                ctxt_tiles[pair][64 * h:64 * (h + 1), qs],
                                ctx_ps[h][0:64, :],
                                bct[:],
                            )

                def outproj(qc):
                    for sub in range(QCW // 128):
                        q0 = qc * QCW + sub * 128
                        for d2 in range(2):
                            ops = pp.tile([128, 512], F32, tag="qkv", bufs=qkv_bufs)
                            for pair in range(2):
                                nc.tensor.matmul(
                                    ops[:],
                                    ctxt_tiles[pair][:, q0:q0 + 128],
                                    wo_t[:, pair, d2 * 512:(d2 + 1) * 512],
                                    start=(pair == 0),
                                    stop=(pair == 1),
                                )
                            osb = op.tile([128, 512], F32, tag="osb")
                            nc.vector.tensor_copy(osb[:], ops[:])
                            nc.sync.dma_start(out_d[q0:q0 + 128, d2 * 512:(d2 + 1) * 512], osb[:])

                # ---- schedule: pair-0 kt/qt first so ACT (exp) starts
                # early; V projection rides in the first attention block.
                for qc in range(QC):
                    kt_proj(0, qc)
                qt_proj(0, 0)
                for st in range(0, 4):
                    v_proj(st)
                attention(0, 0, vproj_from=4, vproj_n=12)
                for qc in range(1, QC):
                    qt_proj(0, qc)
                    attention(0, qc)
                for qc in range(QC):
                    kt_proj(1, qc)
                qt_proj(1, 0)
                attention(1, 0)
                outproj(0)
                for qc in range(1, QC):
                    qt_proj(1, qc)
                    attention(1, qc)
                    outproj(qc)

    nc.compile()
    return nc


def _get_nc(repeat=1):
    key = repeat
    if key not in _CACHE:
        _CACHE[key] = _build(repeat)
    return _CACHE[key]


def _make_in_maps(query_input, Wq, bq, Wk, Wv, Wo):
    import ml_dtypes

    BF = ml_dtypes.bfloat16
    x = np.asarray(query_input, dtype=np.float32)
    in_maps = []
    for core in range(NCORES):
        b, g = divmod(core, NCORES // B)
        cs = slice(g * HPC * HD, (g + 1) * HPC * HD)
        in_maps.append({
            "xt": np.ascontiguousarray(x[b].T.astype(BF)),
            "wq": np.ascontiguousarray(Wq[:, cs].astype(BF)),
            "wk": np.ascontiguousarray(Wk[:, cs].astype(BF)),
            "wv": np.ascontiguousarray(Wv[:, cs].astype(BF)),
            "wo": np.ascontiguousarray(Wo[cs, :].astype(BF)),
            "bq2": np.ascontiguousarray(bq[cs].reshape(2, 128).T.astype(np.float32)),
        })
    return in_maps


def kernel(query_input, Wq, bq, Wk, bk, Wv, bv, Wo, bo):
    from concourse.bass_utils import run_bass_kernel_spmd

    Wq = np.asarray(Wq, np.float32)
    Wk = np.asarray(Wk, np.float32)
    Wv = np.asarray(Wv, np.float32)
    Wo = np.asarray(Wo, np.float32)
    bq = np.asarray(bq, np.float32)
    bv = np.asarray(bv, np.float32)
    bo = np.asarray(bo, np.float32)

    nc = _get_nc()
    in_maps = _make_in_maps(query_input, Wq, bq, Wk, Wv, Wo)
    res = run_bass_kernel_spmd(nc, in_maps, core_ids=list(range(NCORES)))

    gpc = NCORES // B  # groups per batch
    out = np.zeros((B, S, D), np.float32)
    for core in range(NCORES):
        b = core // gpc
        out[b] += res.results[core]["out_p"]
    # bv correction (exact) + bo, applied once on the full output
    out += (bv @ Wo + bo)[None, None, :]
    return out


# revision 7
# speedup vs baseline: 1.2467x; 1.2467x over previous
"""Self-contained 8-core Trainium2 Bass kernel for MultiHeadAttention.

Problem: B=2, S=2048, D=1024, H=16 heads (hd=64), f32, self-attention
(no mask), eval mode (dropout = identity).

Sharding: data-parallel over B (2) x tensor-parallel over heads (4 groups
of 4 heads) = 8 cores. Each core computes, for its batch b and its 4
heads: Q/K/V projections (column-sliced), attention, and a partial
output projection (row-sliced Wo). Host sums the 4 partials per batch
and adds the (bv @ Wo + bo) correction (bv never enters the kernel:
ctx rows sum probs to 1, so (ctx+bv) @ Wo = ctx @ Wo + bv @ Wo).

Algebraic simplifications used (exact):
  - bk dropped: softmax over k is invariant to the per-q constant Q.bk.
  - softmax computed without max subtraction (scores bounded ~|s|<10,
    exp is safe in f32).
  - bq folded into Q^T as a per-partition bias.
  - row normalization deferred past the P@V matmul (scale ctx instead
    of probs); row sums obtained free via an appended ones-column in V.

Performance design (v3):
  - all matmul operands bf16 (f32r streams at 0.5 col/cycle, bf16 at
    1 col/cycle); PSUM accumulation stays f32. Softmax numerator and
    denominator share the bf16 exp values so normalization error
    largely cancels.
  - scores per head-pair as two concurrent K=64 row-tiled matmuls
    (tile_position (0,0)/(64,0)).
  - ACT exp (128 x [128,1024] tiles ~ 142us) is the bottleneck, so the
    emission is software-pipelined around the scores->exp stream: PV
    matmuls, projections, normalizations and the output projection are
    drained from a pending queue in the PE-slack of each exp step.
  - PSUM budget (8 banks): sreg [128,1024] x2 = 4, ctx/outproj shared
    ring [128,512] x3 = 3, proj ring [128,512] x1 = 1.
  - host pre-arranges xt/weights so every DMA is a contiguous
    per-partition block (9 DMAs total); PE warmup matmuls + ACT table
    preload run during the DMA fill.
"""

import sys

sys.path.insert(0, "/opt/trn_rl_repo")

import numpy as np

B, S, D, H, HD = 2, 2048, 1024, 16, 64
HPC = 4  # heads per core
NCORES = 8
DC = D // 128  # 8 contraction chunks
ST = S // 128  # 16 s-tiles
QCW = 512  # q chunk width
QC = S // QCW  # 4 q chunks
KT = S // 128  # 16 k tiles

_CACHE = {}


def _build(repeat=1, ep_bufs=28, target=1.00):
    from collections import deque

    import concourse.bass as bass  # noqa: F401
    import concourse.mybir as mybir
    import concourse.tile as tile
    from concourse import bacc
    from concourse.library_config import attn as attn_lib

    F32 = mybir.dt.float32
    BF16 = mybir.dt.bfloat16
    AF = mybir.ActivationFunctionType

    nc = bacc.Bacc("TRN2", target_bir_lowering=False, debug=False)

    # host pre-arranged layouts (see _make_in_maps)
    xt_d = nc.dram_tensor("xtq", [128, QC, DC, QCW], BF16, kind="ExternalInput")
    wq_d = nc.dram_tensor("wqa", [128, DC, HPC * HD], BF16, kind="ExternalInput")
    wk_d = nc.dram_tensor("wka", [128, DC, HPC * HD], BF16, kind="ExternalInput")
    wv_d = nc.dram_tensor("wva", [128, DC, HPC * HD], BF16, kind="ExternalInput")
    wo_d = nc.dram_tensor("woa", [128, 2, D], BF16, kind="ExternalInput")
    bq_d = nc.dram_tensor("bq2", [128, 2], F32, kind="ExternalInput")
    out_d = nc.dram_tensor("out_p", [S, D], F32, kind="ExternalOutput")

    with tile.TileContext(nc) as tc:
        nc.gpsimd.load_library(attn_lib)
        with (
            tc.tile_pool(name="wp", bufs=1) as wp,
            tc.tile_pool(name="xp", bufs=1) as xp,
            tc.tile_pool(name="qk", bufs=1) as qk,
            tc.tile_pool(name="vp", bufs=1) as vp,
            tc.tile_pool(name="ep", bufs=ep_bufs) as ep,
            tc.tile_pool(name="cp", bufs=1) as cp,
            tc.tile_pool(name="mp", bufs=2) as mp,
            tc.tile_pool(name="op", bufs=3) as op,
            tc.tile_pool(name="pp", bufs=2, space="PSUM") as pp,
        ):
            ones_f = wp.tile([128, 64], BF16, tag="onesf")
            nc.vector.memset(ones_f[:], 1.0)
            scrap = wp.tile([128, 8], BF16, tag="scrap")

            # ---- loads (contiguous per-partition blocks; kt/qt(0) first)
            wk_t = wp.tile([128, DC, HPC * HD], BF16, tag="wk")
            nc.sync.dma_start(wk_t[:], wk_d[:])
            wq_t = wp.tile([128, DC, HPC * HD], BF16, tag="wq")
            nc.sync.dma_start(wq_t[:], wq_d[:])
            bq_t = wp.tile([128, 2], F32, tag="bq")
            nc.sync.dma_start(bq_t[:], bq_d[:])
            xt_t = xp.tile([128, QC, DC, QCW], BF16, tag="xt")
            nc.sync.dma_start(xt_t[:, 0], xt_d[:, 0])
            wv_t = wp.tile([128, DC, HPC * HD], BF16, tag="wv")
            nc.sync.dma_start(wv_t[:], wv_d[:])
            for qcc in range(1, QC):
                nc.sync.dma_start(xt_t[:, qcc], xt_d[:, qcc])
            wo_t = wp.tile([128, 2, D], BF16, tag="wo")
            nc.sync.dma_start(wo_t[:], wo_d[:])

            import contextlib
            if repeat > 1:
                _engs = [mybir.EngineType.PE, mybir.EngineType.Activation,
                         mybir.EngineType.DVE, mybir.EngineType.SP,
                         mybir.EngineType.Pool]
                rep_ctx = tc.For_i(0, repeat, hint_engines=_engs, staggered_reset=True)
            else:
                rep_ctx = contextlib.nullcontext()
            with rep_ctx:
                # ---- ACT exp-table preload + PE HAM warmup during DMA fill
                nc.scalar.activation(scrap[:, 0:8], ones_f[:, 0:8], AF.Exp)
                warm_ps = pp.tile([64, 64], F32, tag="qkv", bufs=1, name="warm")
                for _w in range(40):
                    nc.tensor.matmul(warm_ps[:], ones_f[:, 0:64], ones_f[:, 0:64],
                                     start=True, stop=True)

                # ---- V accumulator [s, 4*(64+1)] with ones columns
                v1_t = vp.tile([128, ST, HPC * 65], BF16, tag="v1")
                nc.vector.memset(
                    v1_t[:].rearrange("p s (h c) -> p s h c", c=65)[:, :, :, 64], 1.0)

                def v_proj(st):
                    vps = pp.tile([128, HPC * HD], F32, tag="qkv", bufs=1, name="vps")
                    for c in range(DC):
                        nc.tensor.matmul(
                            vps[:],
                            xt_t[:, st // 4, c, (st % 4) * 128:(st % 4 + 1) * 128],
                            wv_t[:, c, :],
                            start=(c == 0),
                            stop=(c == DC - 1),
                        )
                    with nc.allow_low_precision(reason="bf16 matmul operands"):
                        nc.vector.tensor_copy(
                            v1_t[:, st, :].rearrange("p (h c) -> p h c", c=65)[:, :, 0:64],
                            vps[:].rearrange("p (h c) -> p h c", c=64),
                        )

                qt_tiles = [qk.tile([128, S], BF16, tag=f"qt{p}", name=f"qt{p}") for p in range(2)]
                kt_tiles = [qk.tile([128, S], BF16, tag=f"kt{p}", name=f"kt{p}") for p in range(2)]

                def kt_proj(pair, qcc):
                    qs = slice(qcc * QCW, (qcc + 1) * QCW)
                    kps = pp.tile([128, QCW], F32, tag="qkv", bufs=1, name="kps")
                    for c in range(DC):
                        nc.tensor.matmul(
                            kps[:],
                            wk_t[:, c, pair * 128:(pair + 1) * 128],
                            xt_t[:, qcc, c, :],
                            start=(c == 0),
                            stop=(c == DC - 1),
                        )
                    with nc.allow_low_precision(reason="bf16 score operands"):
                        nc.vector.tensor_copy(kt_tiles[pair][:, qs], kps[:])

                def qt_proj(pair, qcc):
                    qs = slice(qcc * QCW, (qcc + 1) * QCW)
                    qps = pp.tile([128, QCW], F32, tag="qkv", bufs=1, name="qps")
                    for c in range(DC):
                        nc.tensor.matmul(
                            qps[:],
                            wq_t[:, c, pair * 128:(pair + 1) * 128],
                            xt_t[:, qcc, c, :],
                            start=(c == 0),
                            stop=(c == DC - 1),
                        )
                    with nc.allow_low_precision(reason="bf16 score operands"):
                        nc.vector.tensor_scalar_add(
                            qt_tiles[pair][:, qs], qps[:], bq_t[:, pair:pair + 1]
                        )

                ctxt_tiles = [cp.tile([128, S], BF16, tag=f"ct{p}", name=f"ct{p}") for p in range(2)]

                # ---- software pipeline ------------------------------------
                # pending: deque of (pe_cost_us, closure) drained in PE slack
                pending = deque()

                def drain(budget):
                    while pending and budget > 0.0:
                        cost, fn = pending.popleft()
                        fn()
                        budget -= cost
                    return budget

                ctx_ps = {}   # (pair, qcc) -> [h0_tile, h1_tile]
                expt_of = {}  # (pair, qcc, r) -> expt tile

                def scores_exp(pair, qcc, r):
                    qs = slice(qcc * QCW, (qcc + 1) * QCW)
                    sreg = pp.tile([128, 2 * QCW], F32, tag="big")
                    expt = ep.tile([128, 2 * QCW], BF16, tag="exp")
                    for h in range(2):
                        nc.tensor.matmul(
                            sreg[:, h * QCW:(h + 1) * QCW],
                            kt_tiles[pair][64 * h:64 * (h + 1), r * 128:(r + 1) * 128],
                            qt_tiles[pair][64 * h:64 * (h + 1), qs],
                            start=True,
                            stop=True,
                            tile_position=(64 * h, 0),
                        )
                    with nc.allow_low_precision(reason="bf16 probs"):
                        nc.scalar.activation(expt[:], sreg[:], AF.Exp, scale=0.125)
                    expt_of[(pair, qcc, r)] = expt

                def pv(pair, qcc, h, r):
                    key = (pair, qcc)
                    if key not in ctx_ps:
                        ctx_ps[key] = [
                            pp.tile([65, QCW], F32, tag="ctx", bufs=3,
                                    name=f"ctx{pair}{qcc}{_h}")
                            for _h in range(2)
                        ]
                    hh = 2 * pair + h
                    expt = expt_of[(pair, qcc, r)]
                    nc.tensor.matmul(
                        ctx_ps[key][h][:],
                        v1_t[:, r, 65 * hh:65 * hh + 65],
                        expt[:, h * QCW:(h + 1) * QCW],
                        start=(r == 0),
                        stop=(r == KT - 1),
                    )
                    if h == 1:  # h1 trails h0, so it is the last reader
                        expt_of.pop((pair, qcc, r), None)

                def norm(pair, qcc, h):
                    qs = slice(qcc * QCW, (qcc + 1) * QCW)
                    cps = ctx_ps[(pair, qcc)][h]
                    rsum = mp.tile([1, QCW], F32, tag="rsum")
                    nc.vector.reciprocal(rsum[:], cps[64:65, :])
                    bct = mp.tile([64, QCW], F32, tag="bc")
                    nc.gpsimd.partition_broadcast(bct[:], rsum[:])
                    with nc.allow_low_precision(reason="bf16 matmul operands"):
                        nc.vector.tensor_mul(
                            ctxt_tiles[pair][64 * h:64 * (h + 1), qs],
                            cps[0:64, :],
                            bct[:],
                        )

                def outproj_block(qcc, sub, d2):
                    q0 = qcc * QCW + sub * 128
                    ops = pp.tile([128, 512], F32, tag="ctx", bufs=3, name="ops")
                    for pair in range(2):
                        nc.tensor.matmul(
                            ops[:],
                            ctxt_tiles[pair][:, q0:q0 + 128],
                            wo_t[:, pair, d2 * 512:(d2 + 1) * 512],
                            start=(pair == 0),
                            stop=(pair == 1),
                        )
                    osb = op.tile([128, 512], F32, tag="osb")
                    nc.vector.tensor_copy(osb[:], ops[:])
                    nc.sync.dma_start(out_d[q0:q0 + 128, d2 * 512:(d2 + 1) * 512], osb[:])

                def push_loop_work(pair, qcc):
                    """Queue PV (h1 trails h0 by 6), then norms."""
                    items = []
                    for r in range(KT):
                        items.append((0.22, (lambda p=pair, q=qcc, r=r: pv(p, q, 0, r))))
                        if r >= 6:
                            items.append((0.22, (lambda p=pair, q=qcc, r=r - 6: pv(p, q, 1, r))))
                    for r in range(KT - 6, KT):
                        items.append((0.22, (lambda p=pair, q=qcc, r=r: pv(p, q, 1, r))))
                    items.append((0.05, (lambda p=pair, q=qcc: norm(p, q, 0))))
                    items.append((0.05, (lambda p=pair, q=qcc: norm(p, q, 1))))
                    if pair == 1:
                        for sub in range(4):
                            for d2 in range(2):
                                items.append((0.45, (lambda q=qcc, s=sub, d=d2:
                                                     outproj_block(q, s, d))))
                    pending.extend(items)

                # hard injections: (loop_index, r) -> list of (cost, fn)
                hard = {}

                def add_hard(li, r, cost, fn):
                    hard.setdefault((li, r), []).append((cost, fn))

                # qt for next qc of same pair; kt/qt for pair 1 spread over
                # pair-0 loops 2,3
                for li, (pair, qcc) in enumerate(
                        [(p, q) for p in range(2) for q in range(QC)]):
                    if qcc < QC - 1:
                        add_hard(li, 10, 1.7, (lambda p=pair, q=qcc + 1: qt_proj(p, q)))
                add_hard(2, 2, 1.7, lambda: kt_proj(1, 0))
                add_hard(2, 6, 1.7, lambda: kt_proj(1, 1))
                add_hard(3, 2, 1.7, lambda: kt_proj(1, 2))
                add_hard(3, 6, 1.7, lambda: kt_proj(1, 3))
                add_hard(3, 13, 1.7, lambda: qt_proj(1, 0))
                # V projection rides in loop 0 (PV of loop 0 is deferred)
                for r in range(KT):
                    add_hard(0, r, 0.86, (lambda st=r: v_proj(st)))

                # ---- prelude
                for qcc in range(QC):
                    kt_proj(0, qcc)
                qt_proj(0, 0)

                # ---- main loops
                for li, (pair, qcc) in enumerate(
                        [(p, q) for p in range(2) for q in range(QC)]):
                    for r in range(KT):
                        budget = target - 0.21
                        for cost, fn in hard.pop((li, r), []):
                            fn()
                            budget -= cost
                        scores_exp(pair, qcc, r)
                        drain(budget)
                    push_loop_work(pair, qcc)

                # ---- drain tail
                while pending:
                    _, fn = pending.popleft()
                    fn()

    nc.compile()
    return nc


def _get_nc(repeat=1):
    key = repeat
    if key not in _CACHE:
        _CACHE[key] = _build(repeat)
    return _CACHE[key]


def _part_major(a):
    """[DC*128, N] -> [128, DC, N] contiguous (partition-major)."""
    n = a.shape[1]
    return np.ascontiguousarray(a.reshape(-1, 128, n).transpose(1, 0, 2))


def _make_in_maps(query_input, Wq, bq, Wk, Wv, Wo):
    import ml_dtypes

    BF = ml_dtypes.bfloat16
    x = np.asarray(query_input, dtype=np.float32)
    in_maps = []
    for core in range(NCORES):
        b, g = divmod(core, NCORES // B)
        cs = slice(g * HPC * HD, (g + 1) * HPC * HD)
        xt = x[b].T.astype(BF)  # [D, S]
        # [D, S] -> [128, QC, DC, QCW]: partition p, quarter qc, chunk c
        xtq = np.ascontiguousarray(
            xt.reshape(DC, 128, QC, QCW).transpose(1, 2, 0, 3))
        in_maps.append({
            "xtq": xtq,
            "wqa": _part_major(Wq[:, cs].astype(BF)),
            "wka": _part_major(Wk[:, cs].astype(BF)),
            "wva": _part_major(Wv[:, cs].astype(BF)),
            "woa": _part_major(Wo[cs, :].astype(BF)),
            "bq2": np.ascontiguousarray(bq[cs].reshape(2, 128).T.astype(np.float32)),
        })
    return in_maps


def kernel(query_input, Wq, bq, Wk, bk, Wv, bv, Wo, bo):
    from concourse.bass_utils import run_bass_kernel_spmd

    Wq = np.asarray(Wq, np.float32)
    Wk = np.asarray(Wk, np.float32)
    Wv = np.asarray(Wv, np.float32)
    Wo = np.asarray(Wo, np.float32)
    bq = np.asarray(bq, np.float32)
    bv = np.asarray(bv, np.float32)
    bo = np.asarray(bo, np.float32)

    nc = _get_nc()
    in_maps = _make_in_maps(query_input, Wq, bq, Wk, Wv, Wo)
    res = run_bass_kernel_spmd(nc, in_maps, core_ids=list(range(NCORES)))

    gpc = NCORES // B  # groups per batch
    out = np.zeros((B, S, D), np.float32)
    for core in range(NCORES):
        b = core // gpc
        out[b] += res.results[core]["out_p"]
    # bv correction (exact) + bo, applied once on the full output
    out += (bv @ Wo + bo)[None, None, :]
    return out


# revision 13
# speedup vs baseline: 1.3497x; 1.0827x over previous
"""Self-contained 8-core Trainium2 Bass kernel for MultiHeadAttention.

Problem: B=2, S=2048, D=1024, H=16 heads (hd=64), f32, self-attention
(no mask), eval mode (dropout = identity).

Sharding: data-parallel over B (2) x tensor-parallel over heads (4 groups
of 4 heads) = 8 cores. Each core computes, for its batch b and its 4
heads: Q/K/V projections (column-sliced), attention, and a partial
output projection (row-sliced Wo). Host sums the 4 partials per batch
and adds the (bv @ Wo + bo) correction (bv never enters the kernel:
ctx rows sum probs to 1, so (ctx+bv) @ Wo = ctx @ Wo + bv @ Wo).

Algebraic simplifications used (exact):
  - bk dropped: softmax over k is invariant to the per-q constant Q.bk.
  - softmax computed without max subtraction (scores bounded ~|s|<10,
    exp is safe in f32).
  - bq folded into Q^T as a per-partition bias.
  - row normalization deferred past the P@V matmul (scale ctx instead
    of probs); row sums obtained free via an appended ones-column in V.

Performance design (v3):
  - all matmul operands bf16 (f32r streams at 0.5 col/cycle, bf16 at
    1 col/cycle); PSUM accumulation stays f32. Softmax numerator and
    denominator share the bf16 exp values so normalization error
    largely cancels.
  - scores per head-pair as two concurrent K=64 row-tiled matmuls
    (tile_position (0,0)/(64,0)).
  - ACT exp (128 x [128,1024] tiles ~ 142us) is the bottleneck, so the
    emission is software-pipelined around the scores->exp stream: PV
    matmuls, projections, normalizations and the output projection are
    drained from a pending queue in the PE-slack of each exp step.
  - PSUM budget (8 banks): sreg [128,1024] x2 = 4, ctx/outproj shared
    ring [128,512] x3 = 3, proj ring [128,512] x1 = 1.
  - host pre-arranges xt/weights so every DMA is a contiguous
    per-partition block (9 DMAs total); PE warmup matmuls + ACT table
    preload run during the DMA fill.
"""

import sys

sys.path.insert(0, "/opt/trn_rl_repo")

import numpy as np

B, S, D, H, HD = 2, 2048, 1024, 16, 64
HPC = 4  # heads per core
NCORES = 8
DC = D // 128  # 8 contraction chunks
ST = S // 128  # 16 s-tiles
QCW = 512  # q chunk width
QC = S // QCW  # 4 q chunks
KT = S // 128  # 16 k tiles

_CACHE = {}


def _build(repeat=1, ep_bufs=28, target=1.00):
    from collections import deque

    import concourse.bass as bass  # noqa: F401
    import concourse.mybir as mybir
    import concourse.tile as tile
    from concourse import bacc
    from concourse.library_config import attn as attn_lib

    F32 = mybir.dt.float32
    BF16 = mybir.dt.bfloat16
    AF = mybir.ActivationFunctionType

    nc = bacc.Bacc("TRN2", target_bir_lowering=False, debug=False)

    # host pre-arranged layouts (see _make_in_maps)
    xt_d = nc.dram_tensor("xtq", [128, QC, DC, QCW], BF16, kind="ExternalInput")
    wq_d = nc.dram_tensor("wqa", [128, DC, HPC * HD], BF16, kind="ExternalInput")
    wk_d = nc.dram_tensor("wka", [128, DC, HPC * HD], BF16, kind="ExternalInput")
    wv_d = nc.dram_tensor("wva", [128, DC, HPC * HD], BF16, kind="ExternalInput")
    wo_d = nc.dram_tensor("woa", [128, 2, D], BF16, kind="ExternalInput")
    bq_d = nc.dram_tensor("bq2", [128, 2], F32, kind="ExternalInput")
    out_d = nc.dram_tensor("out_p", [S, D], F32, kind="ExternalOutput")

    with tile.TileContext(nc) as tc:
        nc.gpsimd.load_library(attn_lib)
        with (
            tc.tile_pool(name="wp", bufs=1) as wp,
            tc.tile_pool(name="xp", bufs=1) as xp,
            tc.tile_pool(name="qk", bufs=1) as qk,
            tc.tile_pool(name="vp", bufs=1) as vp,
            tc.tile_pool(name="ep", bufs=ep_bufs) as ep,
            tc.tile_pool(name="cp", bufs=1) as cp,
            tc.tile_pool(name="mp", bufs=2) as mp,
            tc.tile_pool(name="op", bufs=3) as op,
            tc.tile_pool(name="pp", bufs=2, space="PSUM") as pp,
        ):
            ones_f = wp.tile([128, 64], BF16, tag="onesf")
            nc.vector.memset(ones_f[:], 1.0)
            scrap = wp.tile([128, 8], BF16, tag="scrap")

            # ---- loads (contiguous per-partition blocks; kt/qt(0) first)
            wk_t = wp.tile([128, DC, HPC * HD], BF16, tag="wk")
            nc.sync.dma_start(wk_t[:], wk_d[:])
            wq_t = wp.tile([128, DC, HPC * HD], BF16, tag="wq")
            nc.sync.dma_start(wq_t[:], wq_d[:])
            bq_t = wp.tile([128, 2], F32, tag="bq")
            nc.sync.dma_start(bq_t[:], bq_d[:])
            xt_t = xp.tile([128, QC, DC, QCW], BF16, tag="xt")
            nc.sync.dma_start(xt_t[:, 0], xt_d[:, 0])
            wv_t = wp.tile([128, DC, HPC * HD], BF16, tag="wv")
            nc.sync.dma_start(wv_t[:], wv_d[:])
            for qcc in range(1, QC):
                nc.sync.dma_start(xt_t[:, qcc], xt_d[:, qcc])
            wo_t = wp.tile([128, 2, D], BF16, tag="wo")
            nc.sync.dma_start(wo_t[:], wo_d[:])

            import contextlib
            if repeat > 1:
                _engs = [mybir.EngineType.PE, mybir.EngineType.Activation,
                         mybir.EngineType.DVE, mybir.EngineType.SP,
                         mybir.EngineType.Pool]
                rep_ctx = tc.For_i(0, repeat, hint_engines=_engs, staggered_reset=True)
            else:
                rep_ctx = contextlib.nullcontext()
            with rep_ctx:
                # ---- ACT exp-table preload + PE HAM warmup during DMA fill
                nc.scalar.activation(scrap[:, 0:8], ones_f[:, 0:8], AF.Exp)
                warm_ps = pp.tile([64, 64], F32, tag="qkv", bufs=1, name="warm")
                for _w in range(40):
                    nc.tensor.matmul(warm_ps[:], ones_f[:, 0:64], ones_f[:, 0:64],
                                     start=True, stop=True)

                # ---- V accumulator [s, 4*(64+1)] with ones columns
                v1_t = vp.tile([128, ST, HPC * 65], BF16, tag="v1")
                nc.vector.memset(
                    v1_t[:].rearrange("p s (h c) -> p s h c", c=65)[:, :, :, 64], 1.0)

                def v_proj(st):
                    vps = pp.tile([128, HPC * HD], F32, tag="qkv", bufs=1, name="vps")
                    for c in range(DC):
                        nc.tensor.matmul(
                            vps[:],
                            xt_t[:, st // 4, c, (st % 4) * 128:(st % 4 + 1) * 128],
                            wv_t[:, c, :],
                            start=(c == 0),
                            stop=(c == DC - 1),
                        )
                    with nc.allow_low_precision(reason="bf16 matmul operands"):
                        nc.vector.tensor_copy(
                            v1_t[:, st, :].rearrange("p (h c) -> p h c", c=65)[:, :, 0:64],
                            vps[:].rearrange("p (h c) -> p h c", c=64),
                        )

                qt_tiles = [qk.tile([128, S], BF16, tag=f"qt{p}", name=f"qt{p}") for p in range(2)]
                kt_tiles = [qk.tile([128, S], BF16, tag=f"kt{p}", name=f"kt{p}") for p in range(2)]

                _proj_ps = {}

                def _proj(w_t, pair, qcc, cs):
                    """Half of a K/Q projection (contraction chunks cs);
                    both halves share one PSUM tile."""
                    key = (w_t.name, pair, qcc)
                    if key not in _proj_ps:
                        _proj_ps[key] = pp.tile([128, QCW], F32, tag="qkv",
                                                bufs=1, name="prps")
                    prps = _proj_ps[key]
                    for c in cs:
                        nc.tensor.matmul(
                            prps[:],
                            w_t[:, c, pair * 128:(pair + 1) * 128],
                            xt_t[:, qcc, c, :],
                            start=(c == 0),
                            stop=(c == DC - 1),
                        )
                    return prps

                def kt_proj(pair, qcc, cs=range(DC)):
                    kps = _proj(wk_t, pair, qcc, cs)
                    if cs[-1] == DC - 1:
                        qs = slice(qcc * QCW, (qcc + 1) * QCW)
                        with nc.allow_low_precision(reason="bf16 score operands"):
                            nc.vector.tensor_copy(kt_tiles[pair][:, qs], kps[:])

                def qt_proj(pair, qcc, cs=range(DC)):
                    qps = _proj(wq_t, pair, qcc, cs)
                    if cs[-1] == DC - 1:
                        qs = slice(qcc * QCW, (qcc + 1) * QCW)
                        with nc.allow_low_precision(reason="bf16 score operands"):
                            nc.vector.tensor_scalar_add(
                                qt_tiles[pair][:, qs], qps[:], bq_t[:, pair:pair + 1]
                            )

                ctxt_tiles = [cp.tile([128, S], BF16, tag=f"ct{p}", name=f"ct{p}") for p in range(2)]

                # ---- software pipeline ------------------------------------
                # pending: deque of (pe_cost_us, closure) drained in PE slack
                pending = deque()

                def drain(budget):
                    while pending and budget > 0.0:
                        cost, fn = pending.popleft()
                        fn()
                        budget -= cost
                    return budget

                ctx_ps = {}   # (pair, qcc) -> [h0_tile, h1_tile]
                expt_of = {}  # (pair, qcc, r) -> expt tile

                def scores_exp(pair, qcc, r):
                    qs = slice(qcc * QCW, (qcc + 1) * QCW)
                    sreg = pp.tile([128, 2 * QCW], F32, tag="big")
                    expt = ep.tile([128, 2 * QCW], BF16, tag="exp")
                    for h in range(2):
                        nc.tensor.matmul(
                            sreg[:, h * QCW:(h + 1) * QCW],
                            kt_tiles[pair][64 * h:64 * (h + 1), r * 128:(r + 1) * 128],
                            qt_tiles[pair][64 * h:64 * (h + 1), qs],
                            start=True,
                            stop=True,
                            tile_position=(64 * h, 0),
                        )
                    with nc.allow_low_precision(reason="bf16 probs"):
                        nc.scalar.activation(expt[:], sreg[:], AF.Exp, scale=0.125)
                    expt_of[(pair, qcc, r)] = expt

                def pv(pair, qcc, h, r):
                    key = (pair, qcc)
                    if key not in ctx_ps:
                        ctx_ps[key] = [
                            pp.tile([65, QCW], F32, tag="ctx", bufs=3,
                                    name=f"ctx{pair}{qcc}{_h}")
                            for _h in range(2)
                        ]
                    hh = 2 * pair + h
                    expt = expt_of[(pair, qcc, r)]
                    nc.tensor.matmul(
                        ctx_ps[key][h][:],
                        v1_t[:, r, 65 * hh:65 * hh + 65],
                        expt[:, h * QCW:(h + 1) * QCW],
                        start=(r == 0),
                        stop=(r == KT - 1),
                    )
                    if h == 1:  # h1 trails h0, so it is the last reader
                        expt_of.pop((pair, qcc, r), None)

                def norm(pair, qcc, h, sub=None):
                    # sub=None: whole 512-wide chunk; else 128-wide slice
                    w = QCW if sub is None else 128
                    o = 0 if sub is None else sub * 128
                    qs = slice(qcc * QCW + o, qcc * QCW + o + w)
                    cps = ctx_ps[(pair, qcc)][h]
                    rsum = mp.tile([1, QCW], F32, tag="rsum")
                    nc.vector.reciprocal(rsum[:, 0:w], cps[64:65, o:o + w])
                    bct = mp.tile([64, QCW], F32, tag="bc")
                    nc.gpsimd.partition_broadcast(bct[:, 0:w], rsum[:, 0:w])
                    with nc.allow_low_precision(reason="bf16 matmul operands"):
                        nc.vector.tensor_mul(
                            ctxt_tiles[pair][64 * h:64 * (h + 1), qs],
                            cps[0:64, o:o + w],
                            bct[:, 0:w],
                        )

                def outproj_block(qcc, sub, d2):
                    q0 = qcc * QCW + sub * 128
                    ops = pp.tile([128, 512], F32, tag="ctx", bufs=3, name="ops")
                    for pair in range(2):
                        nc.tensor.matmul(
                            ops[:],
                            ctxt_tiles[pair][:, q0:q0 + 128],
                            wo_t[:, pair, d2 * 512:(d2 + 1) * 512],
                            start=(pair == 0),
                            stop=(pair == 1),
                        )
                    osb = op.tile([128, 512], F32, tag="osb")
                    nc.vector.tensor_copy(osb[:], ops[:])
                    nc.sync.dma_start(out_d[q0:q0 + 128, d2 * 512:(d2 + 1) * 512], osb[:])

                def push_loop_work(pair, qcc):
                    """Queue PV (h1 trails h0 by 6) + norms; embed the
                    previous qc's output projection (its norm completed a
                    full loop ago) near the front."""
                    items = []
                    for r in range(KT):
                        items.append((0.22, (lambda p=pair, q=qcc, r=r: pv(p, q, 0, r))))
                        if r >= 6:
                            items.append((0.22, (lambda p=pair, q=qcc, r=r - 6: pv(p, q, 1, r))))
                        if pair == 1 and qcc >= 1 and r == 2:
                            for sub in range(4):
                                for d2 in range(2):
                                    items.append((0.45, (lambda q=qcc - 1, s=sub, d=d2:
                                                         outproj_block(q, s, d))))
                    last = (pair == 1 and qcc == QC - 1)
                    if not last:
                        items.append((0.05, (lambda p=pair, q=qcc: norm(p, q, 0))))
                    for r in range(KT - 6, KT):
                        items.append((0.22, (lambda p=pair, q=qcc, r=r: pv(p, q, 1, r))))
                    if not last:
                        items.append((0.05, (lambda p=pair, q=qcc: norm(p, q, 1))))
                    pending.extend(items)

                # hard injections: (loop_index, r) -> list of (cost, fn)
                hard = {}

                def add_hard(li, r, cost, fn):
                    hard.setdefault((li, r), []).append((cost, fn))

                H1, H2 = range(0, DC // 2), range(DC // 2, DC)

                # qt for next qc of same pair, split in two halves
                for li, (pair, qcc) in enumerate(
                        [(p, q) for p in range(2) for q in range(QC)]):
                    if qcc < QC - 1:
                        add_hard(li, 9, 0.85, (lambda p=pair, q=qcc + 1: qt_proj(p, q, H1)))
                        add_hard(li, 11, 0.85, (lambda p=pair, q=qcc + 1: qt_proj(p, q, H2)))
                # kt/qt for pair 1 spread over pair-0 loops 2,3
                add_hard(2, 2, 0.85, lambda: kt_proj(1, 0, H1))
                add_hard(2, 4, 0.85, lambda: kt_proj(1, 0, H2))
                add_hard(2, 6, 0.85, lambda: kt_proj(1, 1, H1))
                add_hard(2, 8, 0.85, lambda: kt_proj(1, 1, H2))
                add_hard(3, 2, 0.85, lambda: kt_proj(1, 2, H1))
                add_hard(3, 4, 0.85, lambda: kt_proj(1, 2, H2))
                add_hard(3, 6, 0.85, lambda: kt_proj(1, 3, H1))
                add_hard(3, 8, 0.85, lambda: kt_proj(1, 3, H2))
                add_hard(3, 13, 0.85, lambda: qt_proj(1, 0, H1))
                add_hard(3, 14, 0.85, lambda: qt_proj(1, 0, H2))
                # loop 0: V projection + JIT kt(0,1..3) as the xt DMA lands
                vq = 0
                for r in range(KT):
                    if r in (3, 7, 11):
                        add_hard(0, r, 1.7, (lambda q=r // 4 + 1: kt_proj(0, q)))
                    else:
                        add_hard(0, r, 0.86, (lambda st=vq: v_proj(st)))
                        vq += 1
                for j in range(3):
                    add_hard(1, j, 0.86, (lambda st=13 + j: v_proj(st)))

                # ---- prelude
                kt_proj(0, 0)
                qt_proj(0, 0)

                # ---- main loops
                for li, (pair, qcc) in enumerate(
                        [(p, q) for p in range(2) for q in range(QC)]):
                    for r in range(KT):
                        budget = target - 0.21
                        for cost, fn in hard.pop((li, r), []):
                            fn()
                            budget -= cost
                        scores_exp(pair, qcc, r)
                        drain(budget)
                    push_loop_work(pair, qcc)

                # ---- drain tail; last loop's norms split per 128-wide
                # sub-chunk, pipelined with its output projection
                while pending:
                    _, fn = pending.popleft()
                    fn()
                for sub in range(4):
                    norm(1, QC - 1, 0, sub=sub)
                    norm(1, QC - 1, 1, sub=sub)
                    outproj_block(QC - 1, sub, 0)
                    outproj_block(QC - 1, sub, 1)

    nc.compile()
    return nc


def _get_nc(repeat=1):
    key = repeat
    if key not in _CACHE:
        _CACHE[key] = _build(repeat)
    return _CACHE[key]


def _part_major(a):
    """[DC*128, N] -> [128, DC, N] contiguous (partition-major)."""
    n = a.shape[1]
    return np.ascontiguousarray(a.reshape(-1, 128, n).transpose(1, 0, 2))


def _make_in_maps(query_input, Wq, bq, Wk, Wv, Wo):
    import ml_dtypes

    BF = ml_dtypes.bfloat16
    x = np.asarray(query_input, dtype=np.float32)
    in_maps = []
    for core in range(NCORES):
        b, g = divmod(core, NCORES // B)
        cs = slice(g * HPC * HD, (g + 1) * HPC * HD)
        xt = x[b].T.astype(BF)  # [D, S]
        # [D, S] -> [128, QC, DC, QCW]: partition p, quarter qc, chunk c
        xtq = np.ascontiguousarray(
            xt.reshape(DC, 128, QC, QCW).transpose(1, 2, 0, 3))
        in_maps.append({
            "xtq": xtq,
            "wqa": _part_major(Wq[:, cs].astype(BF)),
            "wka": _part_major(Wk[:, cs].astype(BF)),
            "wva": _part_major(Wv[:, cs].astype(BF)),
            "woa": _part_major(Wo[cs, :].astype(BF)),
            "bq2": np.ascontiguousarray(bq[cs].reshape(2, 128).T.astype(np.float32)),
        })
    return in_maps


def kernel(query_input, Wq, bq, Wk, bk, Wv, bv, Wo, bo):
    from concourse.bass_utils import run_bass_kernel_spmd

    Wq = np.asarray(Wq, np.float32)
    Wk = np.asarray(Wk, np.float32)
    Wv = np.asarray(Wv, np.float32)
    Wo = np.asarray(Wo, np.float32)
    bq = np.asarray(bq, np.float32)
    bv = np.asarray(bv, np.float32)
    bo = np.asarray(bo, np.float32)

    nc = _get_nc()
    in_maps = _make_in_maps(query_input, Wq, bq, Wk, Wv, Wo)
    res = run_bass_kernel_spmd(nc, in_maps, core_ids=list(range(NCORES)))

    gpc = NCORES // B  # groups per batch
    out = np.zeros((B, S, D), np.float32)
    for core in range(NCORES):
        b = core // gpc
        out[b] += res.results[core]["out_p"]
    # bv correction (exact) + bo, applied once on the full output
    out += (bv @ Wo + bo)[None, None, :]
    return out
